# revision 1
# baseline (speedup 1.0000x reference)
"""MDTA-style dense attention (B=2, N=4096+8 summary tokens, C=192, H=8, D=24)
on 8 Trainium2 NeuronCores.

Sharding: data-parallel over batch B (2) x tensor-parallel over heads
(4 groups of 2 heads) -> 8 cores, each core computes attention for one batch
and two heads, plus its slice of the qkv projection and the output
projection partial sum (Megatron row-parallel). Partial sums over head
groups are reduced on the host during unsharding.

Device algorithm per core (all layouts transposed: feature-major):
  - qkv projection: q,k in [d, n] layout; v in [m, d] layout (V_aug with an
    appended ones column so the PV matmul also yields the softmax
    denominator).
  - S^T blocks = k_blk^T q (contraction over d on partitions), exp via
    ScalarE (no max-subtraction: |S| <= ~30 so fp32 exp is safe), then
    PV accumulation over key blocks in PSUM.
  - normalize by the denominator row (reciprocal + GPSIMD partition
    broadcast + DVE multiply), project with Wout slice, DMA the transposed
    partial output.
Keys are zero-padded 4104 -> 4224 (33*128); padded keys produce S=0 ->
exp=1 but multiply V_aug rows that are zero (including the ones column),
so they contribute nothing.
"""

import numpy as np

import concourse.bass as bass
import concourse.tile as tile
from concourse import bacc, mybir
from concourse.bass_utils import run_bass_kernel_spmd

# Problem constants (hardcoded per contract).
B = 2
N = 4096          # output tokens
K_SUM = 8         # summary tokens
NT = N + K_SUM    # 4104 total tokens
NP = 4224         # padded key count = 33 * 128
C = 192
H = 8
D = 24
NCORES = 8

CI = 512          # query chunk (8 chunks over 4096)
MB = 128          # key block
GROUP = 3         # key blocks per exp group (3 PSUM banks)
NCHUNKS = N // CI            # 8
MBLOCKS = NP // MB           # 33
NGROUPS = (MBLOCKS + GROUP - 1) // GROUP  # 11

F32 = mybir.dt.float32
F32R = mybir.dt.float32r
F16 = mybir.dt.float16
BF16 = mybir.dt.bfloat16

_CACHED = {}


def _r(ap):
    """View an fp32 AP as float32r for fast PE streaming."""
    return ap.bitcast(F32R)


def build_program():
    nc = bacc.Bacc("TRN2", target_bir_lowering=False, debug=False,
                   num_devices=NCORES)
    xt_d = nc.dram_tensor("XT", [C + 1, NP], F16, kind="ExternalInput")
    wt_d = nc.dram_tensor("WT", [C + 1, 160], F16, kind="ExternalInput")
    wo_d = nc.dram_tensor("WoT", [64, C], F16, kind="ExternalInput")
    out_d = nc.dram_tensor("outT", [C, N], F16, kind="ExternalOutput")

    with tile.TileContext(nc) as tc:
        with tc.tile_pool(name="singles", bufs=1) as singles:
            xt0 = singles.tile([128, NP], F16, tag="xt0")
            xt1 = singles.tile([65, NP], F16, tag="xt1")
            wt0 = singles.tile([128, 160], F16, tag="wt0")
            wt1 = singles.tile([65, 160], F16, tag="wt1")
            wo = singles.tile([32, 2, C], F16, tag="wo")
            qks = singles.tile([128, NP], F16, tag="qks")
            # 4-strip replicas for row-tiled S matmuls (K=24 uses only a
            # 32-row slice of the PE array; 4 strips run concurrently)
            q4 = [singles.tile([128, N], F16, tag=f"q4_{h}", name=f"q4_{h}")
                  for h in range(2)]
            k4 = [singles.tile([128, 9 * MB], F16, tag=f"k4_{h}",
                               name=f"k4_{h}") for h in range(2)]
            vaug = singles.tile([128, MBLOCKS, 64], BF16, tag="vaug")
            otn = [singles.tile([32, N], F16, tag=f"otn{h}", name=f"otn{h}")
                   for h in range(2)]

            # Input loads (host supplies fp16). Weights first: the first
            # production matmul needs them, so they must not queue behind
            # the large XT transfers. XT is chunked so compute starts early.
            nc.sync.dma_start(out=wt0[:], in_=wt_d[0:128, :])
            nc.sync.dma_start(out=wt1[:], in_=wt_d[128:193, :])
            nc.sync.dma_start(out=wo[:, 0, :], in_=wo_d[0:32, :])
            nc.sync.dma_start(out=wo[:, 1, :], in_=wo_d[32:64, :])
            for c0 in range(0, NP, 1056):
                nc.sync.dma_start(out=xt0[:, c0:c0 + 1056],
                                  in_=xt_d[0:128, c0:c0 + 1056])
                nc.gpsimd.dma_start(out=xt1[:, c0:c0 + 1056],
                                    in_=xt_d[128:193, c0:c0 + 1056])

            xts = (xt0, xt1)
            wts = (wt0, wt1)

            # ---- q/k production: 4 roles col-tiled into one PSUM bank,
            # running concurrently on separate 32-column PE strips. Role r
            # lands at partitions 32r, matching the strip layout directly.
            with tc.tile_pool(name="qkpsum", bufs=4, space="PSUM") as qkp:
                for ci in range(9):
                    c0 = ci * CI
                    w = CI if ci < 8 else MB   # last chunk: cols 4096:4224
                    ps = qkp.tile([128, CI], F32, tag="qk")
                    for r in range(4):         # q_h0, q_h1, k_h0, k_h1
                        if ci == 8 and r < 2:
                            continue  # q only needs 4096 cols
                        for kc in range(2):
                            nc.tensor.matmul(
                                ps[32 * r:32 * r + D, :w],
                                lhsT=wts[kc][:, 24 * r:24 * r + D],
                                rhs=xts[kc][:, c0:c0 + w],
                                start=(kc == 0), stop=(kc == 1),
                                tile_position=(0, 32 * r),
                                skip_group_check=True)
                    nc.vector.tensor_copy(out=qks[:, c0:c0 + w],
                                          in_=ps[:, :w])

            # replicate q into 4 partition strips; scatter k blocks
            # round-robin over strips (block mb -> strip mb%4, col mb//4)
            for h in range(2):
                q_src = qks[32 * h:32 * h + D, :]
                k_src = qks[64 + 32 * h:64 + 32 * h + D, :]
                for st in range(4):
                    eng = nc.sync if st % 2 == 0 else nc.gpsimd
                    for cq in range(0, N, 1024):
                        eng.dma_start(
                            out=q4[h][32 * st:32 * st + D, cq:cq + 1024],
                            in_=q_src[:, cq:cq + 1024])
                for mb in range(MBLOCKS):
                    st, t = mb % 4, mb // 4
                    eng = nc.sync if st % 2 == 0 else nc.gpsimd
                    eng.dma_start(
                        out=k4[h][32 * st:32 * st + D, t * MB:(t + 1) * MB],
                        in_=k_src[:, mb * MB:(mb + 1) * MB])

            # ---- V_aug production: [m, d] layout via per-block matmuls ----
            with tc.tile_pool(name="vpsum", bufs=4, space="PSUM") as vps:
                # V_aug per-head 32-col strip: [ones-indicator, v (24), 0*7].
                # The indicator feature row of XT makes the matmul emit the
                # ones column (and zeros for padded keys) directly.
                for mb in range(MBLOCKS):
                    m0 = mb * MB
                    ps = vps.tile([128, 64], F32, tag="v")
                    for kc in range(2):
                        nc.tensor.matmul(
                            ps[:],
                            lhsT=xts[kc][:, m0:m0 + MB],
                            rhs=wts[kc][:, 96:160],
                            start=(kc == 0), stop=(kc == 1))
                    nc.vector.tensor_copy(out=vaug[:, mb, :], in_=ps[:])

            # ---- attention ----
            with (tc.tile_pool(name="spsum", bufs=2, space="PSUM") as sp,
                  tc.tile_pool(name="opsum", bufs=2, space="PSUM") as op,
                  tc.tile_pool(name="exps", bufs=5) as ep,
                  tc.tile_pool(name="bcast", bufs=4) as bp):
                for ci in range(NCHUNKS):
                    c0 = ci * CI
                    for h in range(2):
                        o_ps = op.tile([32, CI], F32, tag="o")
                        for g in range(NGROUPS):
                            nblk = min(GROUP, MBLOCKS - g * GROUP)
                            s_ps = sp.tile([128, GROUP, CI], F32, tag="s")
                            for j in range(nblk):
                                mb = g * GROUP + j
                                st, t = mb % 4, mb // 4
                                p0 = 32 * st
                                nc.tensor.matmul(
                                    s_ps[:, j, :],
                                    lhsT=k4[h][p0:p0 + D, t * MB:(t + 1) * MB],
                                    rhs=q4[h][p0:p0 + D, c0:c0 + CI],
                                    start=True, stop=True,
                                    tile_position=(p0, 0))
                            e_t = ep.tile([128, GROUP, CI], BF16, tag="e")
                            nc.scalar.activation(
                                out=e_t[:, 0:nblk, :], in_=s_ps[:, 0:nblk, :],
                                func=mybir.ActivationFunctionType.Exp)
                            for j in range(nblk):
                                mb = g * GROUP + j
                                nc.tensor.matmul(
                                    o_ps[:, :],
                                    lhsT=vaug[:, mb, 32 * h:32 * h + 32],
                                    rhs=e_t[:, j, :],
                                    start=(mb == 0), stop=(mb == MBLOCKS - 1))
                        # normalize: row 0 is the denominator (ones column is
                        # first in V_aug). reciprocal -> broadcast over the
                        # 32-row strip -> multiply while copying PSUM->SBUF.
                        rc = bp.tile([32, CI], F32, tag="rc")
                        nc.vector.reciprocal(out=rc[0:1, :], in_=o_ps[0:1, :])
                        bc = bp.tile([32, CI], F32, tag="bc")
                        nc.gpsimd.partition_broadcast(bc[:, :], rc[0:1, :])
                        nc.vector.tensor_mul(
                            out=otn[h][:, c0:c0 + CI],
                            in0=o_ps[:, :], in1=bc[:, :])

            # ---- output projection (row-parallel partial; host sums groups) ----
            with (tc.tile_pool(name="proja", bufs=4, space="PSUM") as pa,
                  tc.tile_pool(name="projb", bufs=4, space="PSUM") as pb,
                  tc.tile_pool(name="projsb", bufs=4) as psb):
                for ci in range(NCHUNKS):
                    c0 = ci * CI
                    t_a = pa.tile([128, CI], F32, tag="pa")
                    t_b = pb.tile([64, CI], F32, tag="pb")
                    for h in range(2):
                        nc.tensor.matmul(t_a[:], lhsT=wo[:, h, 0:128],
                                         rhs=otn[h][:, c0:c0 + CI],
                                         start=(h == 0), stop=(h == 1))
                        nc.tensor.matmul(t_b[:], lhsT=wo[:, h, 128:192],
                                         rhs=otn[h][:, c0:c0 + CI],
                                         start=(h == 0), stop=(h == 1))
                    s_a = psb.tile([128, CI], F16, tag="sa")
                    s_b = psb.tile([64, CI], F16, tag="sb")
                    nc.vector.tensor_copy(out=s_a[:], in_=t_a[:])
                    nc.vector.tensor_copy(out=s_b[:], in_=t_b[:])
                    nc.sync.dma_start(out=out_d[0:128, c0:c0 + CI], in_=s_a[:])
                    nc.sync.dma_start(out=out_d[128:192, c0:c0 + CI], in_=s_b[:])

    nc.compile()
    return nc


def make_in_maps(X_flat, S_tokens, Wqkv, Wout, temperature):
    temp = np.asarray(temperature, dtype=np.float32).reshape(H)
    Wq = np.asarray(Wqkv[0:C], dtype=np.float32)
    Wk = np.asarray(Wqkv[C:2 * C], dtype=np.float32)
    Wv = np.asarray(Wqkv[2 * C:3 * C], dtype=np.float32)
    Wout = np.asarray(Wout, dtype=np.float32)

    xts = []
    for b in range(B):
        x_in = np.concatenate([np.asarray(X_flat[b], dtype=np.float32),
                               np.asarray(S_tokens[b], dtype=np.float32)], axis=0)
        xt = np.zeros((C + 1, NP), dtype=np.float32)
        xt[:C, :NT] = np.ascontiguousarray(x_in.T)
        xt[C, :NT] = 1.0  # indicator feature -> ones column of V_aug
        xts.append(xt)

    in_maps = []
    for core in range(NCORES):
        b = core // 4
        h0 = 2 * (core % 4)
        h1 = h0 + 1
        wt = np.zeros((C + 1, 160), dtype=np.float32)
        wt[:C, 0:24] = (Wq[h0 * D:(h0 + 1) * D] * temp[h0]).T
        wt[:C, 24:48] = (Wq[h1 * D:(h1 + 1) * D] * temp[h1]).T
        wt[:C, 48:72] = Wk[h0 * D:(h0 + 1) * D].T
        wt[:C, 72:96] = Wk[h1 * D:(h1 + 1) * D].T
        wt[C, 96] = 1.0                                   # ones indicator h0
        wt[:C, 97:121] = Wv[h0 * D:(h0 + 1) * D].T
        wt[C, 128] = 1.0                                  # ones indicator h1
        wt[:C, 129:153] = Wv[h1 * D:(h1 + 1) * D].T
        wo = np.zeros((64, C), dtype=np.float32)
        wo[1:25] = Wout[:, h0 * D:(h0 + 1) * D].T
        wo[33:57] = Wout[:, h1 * D:(h1 + 1) * D].T
        in_maps.append({
            "XT": np.ascontiguousarray(xts[b]).astype(np.float16),
            "WT": np.ascontiguousarray(wt).astype(np.float16),
            "WoT": np.ascontiguousarray(wo).astype(np.float16),
        })
    return in_maps


def run(in_maps, **kwargs):
    if "nc" not in _CACHED:
        _CACHED["nc"] = build_program()
    return run_bass_kernel_spmd(_CACHED["nc"], in_maps,
                                core_ids=list(range(NCORES)), **kwargs)


def kernel(X_flat, S_tokens, Wqkv, Wout, temperature):
    in_maps = make_in_maps(X_flat, S_tokens, Wqkv, Wout, temperature)
    res = run(in_maps)
    out = np.zeros((B, N, C), dtype=np.float32)
    for core in range(NCORES):
        out[core // 4] += res.results[core]["outT"].T.astype(np.float32)
    return out

